# revision 3
# baseline (speedup 1.0000x reference)
"""Bottleneck-transformer block on 8 TRN2 NeuronCores — fp8 DoubleRow version.

Sharding: data-parallel over batch (B=64 -> 8 elements/core), weights
replicated; no collectives. All GEMMs run as fp8e4 DoubleRow matmuls
(K=256 per instruction at 0.5 cycles/row): conv1, q/k/v projections,
attention logits (rel-pos term packed into the second DR plane), softmax
column sums (fp8 ones), attn@V, and conv3+shortcut fused into one PSUM
accumulation group. The shortcut GEMM dominates the fp8 error budget, so it
is hi/lo error-compensated (w_hi*x_hi + w_lo*x_hi + w_hi*x_lo, all at one
shared scale). BatchNorms, conv biases, and the attention value bias fold on
the host; q/k biases only shift logits by per-column constants under softmax
and are dropped / merged exactly by using the biased q-hat as the logits rhs
plane. Per-output-channel weight scales dequantize through per-partition
ACT/DVE scale+bias vectors.
"""

import numpy as np
import ml_dtypes

import concourse.bass as bass
import concourse.mybir as mybir
from concourse import bacc
from concourse.tile import TileContext
from concourse.bass_utils import run_bass_kernel_spmd

EPS = 1e-5
NCORES = 8
BLOC = 8           # batch elements per core
NT = 256           # tokens per element (16*16)
F32 = mybir.dt.float32
F32R = mybir.dt.float32r
F8 = mybir.dt.float8e4
E4 = ml_dtypes.float8_e4m3
DR = mybir.MatmulPerfMode.DoubleRow
MULT = mybir.AluOpType.mult
ADD = mybir.AluOpType.add

# quantization design constants (input distribution is fixed by the problem)
SX = 16.0          # x activation scale
SO1 = 16.0         # out1 activation scale
SQ = 32.0          # q scale
SKP = 32.0         # k / rel-pos shared scale
SO2 = 16.0         # out2 activation scale
SAFE = 0.95
MAXV = 224.0
EXPSCALE = 1.0 / (SQ * SKP)
EXPSHIFT = float(np.log(64.0) - 4.85)

_STATE = {}

_F8_SHAPES = {
    "xh": [BLOC, 128, 4, 2, NT], "xl": [BLOC, 128, 4, 2, NT],
    "w1": [128, 4, 2, 512],
    "qw": [128, 2, 2, 512], "kw": [128, 2, 2, 512], "vw": [128, 2, 2, 512],
    "w3": [128, 2, 2, 2048],
    "wsh": [128, 4, 2, 2048], "wsl": [128, 4, 2, 2048],
    "pos": [128, 4, NT],
}
_F32_SHAPES = {
    "b1s": [128, 4], "b1b": [128, 4], "qs": [128, 4], "qb8": [128, 4],
    "ks": [128, 4], "o2s": [128, 4], "o2b": [128, 4],
    "fs": [128, 16], "fb": [128, 16], "uvt": [128, 1],
}


def _build_nc():
    nc = bacc.Bacc("TRN2", target_bir_lowering=False, debug=False,
                   num_devices=NCORES)
    d = {}
    for k, v in _F8_SHAPES.items():
        d[k] = nc.declare_dram_parameter(k, v, F8, isOutput=False)
    for k, v in _F32_SHAPES.items():
        d[k] = nc.declare_dram_parameter(k, v, F32, isOutput=False)
    out_d = nc.declare_dram_parameter("out", [BLOC, 16, 128, NT], F32,
                                      isOutput=True)

    RELU = mybir.ActivationFunctionType.Relu
    EXPF = mybir.ActivationFunctionType.Exp

    def mmdr(ps, lhsT, rhs, start, stop):
        nc.tensor.matmul(ps, lhsT, rhs, start=start, stop=stop, perf_mode=DR)

    with TileContext(nc) as tc:
        with (
            tc.tile_pool(name="wp", bufs=1) as wp,
            tc.tile_pool(name="act", bufs=2) as act,
            tc.tile_pool(name="att", bufs=4) as att,
            tc.tile_pool(name="outp", bufs=4) as outp,
            tc.tile_pool(name="psA", bufs=4, space="PSUM") as psA,
            tc.tile_pool(name="psB", bufs=2, space="PSUM") as psB,
            tc.tile_pool(name="psC", bufs=2, space="PSUM") as psC,
        ):
            W1 = wp.tile([128, 4, 2, 512], F8)
            QW = wp.tile([128, 2, 2, 512], F8)
            KW = wp.tile([128, 2, 2, 512], F8)
            VW = wp.tile([128, 2, 2, 512], F8)
            W3 = wp.tile([128, 2, 2, 2048], F8)
            WSH = wp.tile([128, 4, 2, 2048], F8)
            WSL = wp.tile([128, 4, 2, 2048], F8)
            PR = wp.tile([128, 4, 2, NT], F8)    # plane0: q-hat, plane1: pos
            ON8 = wp.tile([128, 2, 16], F8)
            ONMF = wp.tile([1, 128], F32)
            ONM = wp.tile([1, 128], F32R)
            ESH = wp.tile([128, 1], F32)
            ESC = wp.tile([128, 1], F32)
            B1S = wp.tile([128, 4], F32)
            B1B = wp.tile([128, 4], F32)
            QS = wp.tile([128, 4], F32)
            QB8 = wp.tile([128, 4], F32)
            KS = wp.tile([128, 4], F32)
            O2S = wp.tile([128, 4], F32)
            O2B = wp.tile([128, 4], F32)
            FS = wp.tile([128, 16], F32)
            FB = wp.tile([128, 16], F32)
            UVT = wp.tile([128, 1], F32)

            nc.vector.memset(ON8, 1.0)
            nc.vector.memset(ONMF, 1.0)
            nc.vector.tensor_copy(out=ONM, in_=ONMF)
            nc.vector.memset(ESH, EXPSHIFT)
            nc.vector.memset(ESC, EXPSCALE)

            def load_initial_weights():
                for k, t in [("b1s", B1S), ("b1b", B1B), ("qs", QS),
                             ("qb8", QB8), ("ks", KS), ("o2s", O2S),
                             ("o2b", O2B), ("fs", FS), ("fb", FB),
                             ("uvt", UVT)]:
                    nc.sync.dma_start(out=t, in_=d[k][:])
                nc.sync.dma_start(out=W1, in_=d["w1"][:])
                nc.sync.dma_start(out=QW, in_=d["qw"][:])
                nc.sync.dma_start(out=KW, in_=d["kw"][:])
                nc.sync.dma_start(out=VW, in_=d["vw"][:])
                for h in range(4):
                    nc.sync.dma_start(out=PR[:, h, 1, :], in_=d["pos"][:, h, :])

            def load_big_weights():
                for g in range(4):
                    sl = slice(g * 512, (g + 1) * 512)
                    nc.sync.dma_start(out=W3[:, :, :, sl],
                                      in_=d["w3"][:, :, :, sl])
                    nc.sync.dma_start(out=WSH[:, :, :, sl],
                                      in_=d["wsh"][:, :, :, sl])
                    nc.sync.dma_start(out=WSL[:, :, :, sl],
                                      in_=d["wsl"][:, :, :, sl])

            def body(e):
                XH = act.tile([128, 4, 2, NT], F8, tag="xh", name=f"xh{e}")
                XL = act.tile([128, 4, 2, NT], F8, tag="xl", name=f"xl{e}")
                nc.sync.dma_start(out=XH, in_=d["xh"][e])
                nc.sync.dma_start(out=XL, in_=d["xl"][e])
                if e == 0:
                    load_big_weights()

                # conv1 + bn1 + relu -> O1 fp8 (channels as [jj, i] pairs)
                O1 = act.tile([128, 2, 2, NT], F8, tag="o1", name=f"o1_{e}")
                for m in range(4):
                    ps = psA.tile([128, NT], F32, tag="mm")
                    for jj in range(4):
                        mmdr(ps, W1[:, jj, :, m * 128:(m + 1) * 128],
                             XH[:, jj], jj == 0, jj == 3)
                    nc.scalar.activation(O1[:, m // 2, m % 2, :], ps, RELU,
                                         bias=B1B[:, m:m + 1],
                                         scale=B1S[:, m:m + 1])

                # q projection: raw into KQ plane1, biased q-hat into PR pl0
                KQ = act.tile([128, 4, 2, NT], F8, tag="kq", name=f"kq{e}")
                for h in range(4):
                    ps = psA.tile([128, NT], F32, tag="mm")
                    for jj in range(2):
                        mmdr(ps, QW[:, jj, :, h * 128:(h + 1) * 128],
                             O1[:, jj], jj == 0, jj == 1)
                    nc.vector.tensor_scalar_mul(out=KQ[:, h, 1, :], in0=ps,
                                                scalar1=QS[:, h:h + 1])
                    nc.vector.tensor_scalar(out=PR[:, h, 0, :], in0=ps,
                                            scalar1=QS[:, h:h + 1],
                                            scalar2=QB8[:, h:h + 1],
                                            op0=MULT, op1=ADD)

                # k projection (no bias needed) into KQ plane0
                for h in range(4):
                    ps = psA.tile([128, NT], F32, tag="mm")
                    for jj in range(2):
                        mmdr(ps, KW[:, jj, :, h * 128:(h + 1) * 128],
                             O1[:, jj], jj == 0, jj == 1)
                    nc.vector.tensor_scalar_mul(out=KQ[:, h, 0, :], in0=ps,
                                                scalar1=KS[:, h:h + 1])

                # v, transposed: VT[tok, mt, c] (per-channel svw rides along)
                VT = act.tile([128, 2, 512], F8, tag="vt", name=f"vt{e}")
                for mt in range(2):
                    for cc in range(2):
                        ps = psA.tile([128, NT], F32, tag="mm")
                        for jj in range(2):
                            mmdr(ps, O1[:, jj, :, mt * 128:(mt + 1) * 128],
                                 VW[:, jj, :, cc * 256:(cc + 1) * 256],
                                 jj == 0, jj == 1)
                        nc.vector.tensor_scalar_mul(
                            out=VT[:, mt, cc * 256:(cc + 1) * 256], in0=ps,
                            scalar1=UVT)

                # logits^T + exp: one DR matmul per (h, mt)
                EXT = att.tile([128, 4, 2, NT], F8, tag="ext", bufs=2,
                               name=f"ext{e}")
                for h in range(4):
                    for mt in range(2):
                        psl = psA.tile([128, NT], F32, tag="mm")
                        mmdr(psl, KQ[:, h, :, mt * 128:(mt + 1) * 128],
                             PR[:, h], True, True)
                        nc.scalar.activation(EXT[:, h, mt, :], psl, EXPF,
                                             bias=ESH, scale=ESC)

                # softmax sums (fp8 ones DR matmul) -> 1/sum
                recs = []
                for h in range(4):
                    pss = psC.tile([1, NT], F32, tag="sum")
                    mmdr(pss, ON8[:, :, 0:1], EXT[:, h], True, True)
                    rec = att.tile([1, NT], F32R, tag="rec", bufs=4,
                                   name=f"rec{e}_{h}")
                    with nc.allow_low_precision(reason="softmax 1/sum"):
                        nc.vector.reciprocal(out=rec, in_=pss)
                    recs.append(rec)

                # broadcast 1/sum over partitions (K=1 f32r outer product)
                rcbs = []
                for h in range(4):
                    psr = psB.tile([128, NT], F32, tag="rcb")
                    nc.tensor.matmul(psr, ONM, recs[h], start=True, stop=True)
                    rcb = att.tile([128, NT], F32, tag="rcbs", bufs=4,
                                   name=f"rcb{e}_{h}")
                    nc.vector.tensor_copy(out=rcb, in_=psr)
                    rcbs.append(rcb)

                # attn @ V, normalize, bn2 + relu -> O2 fp8
                O2 = act.tile([128, 2, 2, NT], F8, tag="o2", name=f"o2_{e}")
                for h in range(4):
                    pso = psA.tile([128, NT], F32, tag="mm")
                    mmdr(pso, VT[:, :, h * 128:(h + 1) * 128], EXT[:, h],
                         True, True)
                    tmp = att.tile([128, NT], F32, tag="tmp", bufs=2,
                                   name=f"tmp{e}_{h}")
                    nc.vector.tensor_tensor(out=tmp, in0=pso, in1=rcbs[h],
                                            op=MULT)
                    nc.scalar.activation(O2[:, h // 2, h % 2, :], tmp, RELU,
                                         bias=O2B[:, h:h + 1],
                                         scale=O2S[:, h:h + 1])

                # conv3 + compensated shortcut fused, + relu -> out
                for m in range(16):
                    sl = slice(m * 128, (m + 1) * 128)
                    ps = psA.tile([128, NT], F32, tag="mm")
                    for jj in range(4):
                        mmdr(ps, WSH[:, jj, :, sl], XH[:, jj], jj == 0, False)
                    for jj in range(4):
                        mmdr(ps, WSL[:, jj, :, sl], XH[:, jj], False, False)
                    for jj in range(4):
                        mmdr(ps, WSH[:, jj, :, sl], XL[:, jj], False, False)
                    for jj in range(2):
                        mmdr(ps, W3[:, jj, :, sl], O2[:, jj], False, jj == 1)
                    ot = outp.tile([128, NT], F32, tag="ot")
                    nc.scalar.activation(ot, ps, RELU, bias=FB[:, m:m + 1],
                                         scale=FS[:, m:m + 1])
                    nc.sync.dma_start(out=out_d[e, m], in_=ot)

            load_initial_weights()
            for e in range(BLOC):
                body(e)

    nc.compile()
    return nc


def _q8(x):
    """quantize to fp8e4 values, returned as float32."""
    v = np.asarray(x, np.float32).astype(E4)
    return v.astype(np.float32)


def _r8(wq):
    """quantized [M, K] weight (f32 container) -> lhsT/moving layout
    [128, K//256, 2, M] fp8."""
    m, k = wq.shape
    t = np.ascontiguousarray(
        wq.T.reshape(k // 256, 2, 128, m).transpose(2, 0, 1, 3))
    return t.astype(E4)


def _b(v):
    """[C] vector -> [128, C//128] per-m-tile layout."""
    return np.ascontiguousarray(
        np.asarray(v, np.float64).reshape(-1, 128).T).astype(np.float32)


def _prep_shared(i):
    s1 = (i["bn1_g"] / np.sqrt(i["bn1_v"] + EPS)).astype(np.float64)
    w1f = i["conv1_w"].astype(np.float64) * s1[:, None]
    b1 = i["bn1_b"].astype(np.float64) - i["bn1_m"].astype(np.float64) * s1

    s2 = (i["bn2_g"] / np.sqrt(i["bn2_v"] + EPS)).astype(np.float64)
    b2 = (i["bn2_b"].astype(np.float64)
          - i["bn2_m"].astype(np.float64) * s2
          + s2 * i["v_b"].astype(np.float64))   # v bias folded (probs sum 1)

    s3 = (i["bn3_g"] / np.sqrt(i["bn3_v"] + EPS)).astype(np.float64)
    w3f = i["conv3_w"].astype(np.float64) * s3[:, None]
    b3 = i["bn3_b"].astype(np.float64) - i["bn3_m"].astype(np.float64) * s3

    ss = (i["scbn_g"] / np.sqrt(i["scbn_v"] + EPS)).astype(np.float64)
    wscf = i["sc_w"].astype(np.float64) * ss[:, None]
    bsc = (ss * (i["sc_b"].astype(np.float64)
                 - i["scbn_m"].astype(np.float64))
           + i["scbn_b"].astype(np.float64))

    sw1 = SAFE * MAXV / np.abs(w1f).max(axis=1)
    sqw = SAFE * MAXV / np.abs(i["q_w"]).max(axis=1)
    skw = SAFE * MAXV / np.abs(i["k_w"]).max(axis=1)
    svw = SAFE * MAXV / np.abs(i["v_w"]).max(axis=1)
    c_ch = SAFE * MAXV / np.maximum(np.abs(w3f).max(axis=1) / SO2,
                                    np.abs(wscf).max(axis=1) / SX)
    uv = 32.0 / (SO1 * np.median(svw))

    wscs = wscf * (c_ch / SX)[:, None]
    wsch_f = _q8(wscs)

    pos = (np.asarray(i["rel_h"], np.float64)
           + np.asarray(i["rel_w"], np.float64)).reshape(4, 128, NT)
    pos8 = (pos * SKP).astype(np.float32).astype(E4)

    return {
        "w1": _r8(_q8(w1f * sw1[:, None])),
        "qw": _r8(_q8(i["q_w"] * sqw[:, None])),
        "kw": _r8(_q8(i["k_w"] * skw[:, None])),
        "vw": _r8(_q8(i["v_w"] * svw[:, None])),
        "w3": _r8(_q8(w3f * (c_ch / SO2)[:, None])),
        "wsh": _r8(wsch_f),
        "wsl": _r8(_q8(wscs - wsch_f)),
        "pos": np.ascontiguousarray(pos8.transpose(1, 0, 2)),
        "b1s": _b(1.0 / (sw1 * SX) * SO1), "b1b": _b(b1 * SO1),
        "qs": _b(SQ / (sqw * SO1)), "qb8": _b(SQ * i["q_b"]),
        "ks": _b(SKP / (skw * SO1)),
        "o2s": _b(s2 * SO2 / (svw * SO1 * uv)), "o2b": _b(b2 * SO2),
        "fs": _b(1.0 / c_ch), "fb": _b(b3 + bsc),
        "uvt": np.full((128, 1), uv, np.float32),
    }


def _prep_x(x):
    """full x [64, 1024, 16, 16] -> per-core hi/lo fp8 [8, BLOC,128,4,2,NT]."""
    xs = np.asarray(x, np.float32).reshape(64, 1024, NT) * SX
    xh = xs.astype(E4)
    xl = (xs - xh.astype(np.float32)).astype(E4)

    def lay(a):
        t = a.reshape(64, 4, 2, 128, NT).transpose(0, 3, 1, 2, 4)
        return np.ascontiguousarray(t).reshape(8, BLOC, 128, 4, 2, NT)

    return lay(xh), lay(xl)


def kernel(**inputs):
    if "nc" not in _STATE:
        _STATE["nc"] = _build_nc()
    nc = _STATE["nc"]

    i = {k: np.asarray(v) for k, v in inputs.items()}
    shared = _prep_shared(i)
    xh, xl = _prep_x(i["x"])

    in_maps = []
    for c in range(NCORES):
        m = dict(shared)
        m["xh"] = xh[c]
        m["xl"] = xl[c]
        in_maps.append(m)

    res = run_bass_kernel_spmd(nc, in_maps, list(range(NCORES)))
    out = np.concatenate(
        [res.results[c]["out"].reshape(BLOC, 2048, 16, 16)
         for c in range(NCORES)], axis=0)
    return out.astype(np.float32)


# revision 4
# speedup vs baseline: 1.0647x; 1.0647x over previous
"""Bottleneck-transformer block on 8 TRN2 NeuronCores — fp8 DoubleRow version.

Sharding: data-parallel over batch (B=64 -> 8 elements/core), weights
replicated; no collectives. All GEMMs run as fp8e4 DoubleRow matmuls
(K=256 per instruction at 0.5 cycles/row): conv1, q/k/v projections,
attention logits (rel-pos term packed into the second DR plane), softmax
column sums (fp8 ones), attn@V, and conv3+shortcut fused into one PSUM
accumulation group. The shortcut GEMM dominates the fp8 error budget, so it
is hi/lo error-compensated (w_hi*x_hi + w_lo*x_hi + w_hi*x_lo, all at one
shared scale). BatchNorms, conv biases, and the attention value bias fold on
the host; q/k biases only shift logits by per-column constants under softmax
and are dropped / merged exactly by using the biased q-hat as the logits rhs
plane. Per-output-channel weight scales dequantize through per-partition
ACT/DVE scale+bias vectors.
"""

import numpy as np
import ml_dtypes

import concourse.bass as bass
import concourse.mybir as mybir
from concourse import bacc
from concourse.tile import TileContext
from concourse.bass_utils import run_bass_kernel_spmd

EPS = 1e-5
NCORES = 8
BLOC = 8           # batch elements per core
NT = 256           # tokens per element (16*16)
F32 = mybir.dt.float32
F32R = mybir.dt.float32r
F8 = mybir.dt.float8e4
E4 = ml_dtypes.float8_e4m3
DR = mybir.MatmulPerfMode.DoubleRow
MULT = mybir.AluOpType.mult
ADD = mybir.AluOpType.add

# quantization design constants (input distribution is fixed by the problem)
SX = 16.0          # x activation scale
SO1 = 16.0         # out1 activation scale
SQ = 32.0          # q scale
SKP = 32.0         # k / rel-pos shared scale
SO2 = 16.0         # out2 activation scale
SAFE = 0.95
MAXV = 224.0
EXPSCALE = 1.0 / (SQ * SKP)
EXPSHIFT = float(np.log(64.0) - 4.85)

_STATE = {}

_F8_SHAPES = {
    "xh": [BLOC, 128, 4, 2, NT], "xl": [BLOC, 128, 4, 2, NT],
    "w1": [128, 4, 2, 512],
    "qw": [128, 2, 2, 512], "kw": [128, 2, 2, 512], "vw": [128, 2, 2, 512],
    "w3": [128, 2, 2, 2048],
    "wsh": [128, 4, 2, 2048], "wsl": [128, 4, 2, 2048],
    "pos": [128, 4, NT],
}
_F32_SHAPES = {
    "b1s": [128, 4], "b1b": [128, 4], "qs": [128, 4], "qb8": [128, 4],
    "ks": [128, 4], "o2s": [128, 4], "o2b": [128, 4],
    "fs": [128, 16], "fb": [128, 16], "uvt": [128, 1],
}


def _build_nc():
    nc = bacc.Bacc("TRN2", target_bir_lowering=False, debug=False,
                   num_devices=NCORES)
    d = {}
    for k, v in _F8_SHAPES.items():
        d[k] = nc.declare_dram_parameter(k, v, F8, isOutput=False)
    for k, v in _F32_SHAPES.items():
        d[k] = nc.declare_dram_parameter(k, v, F32, isOutput=False)
    out_d = nc.declare_dram_parameter("out", [BLOC, 16, 128, NT], F32,
                                      isOutput=True)

    RELU = mybir.ActivationFunctionType.Relu
    EXPF = mybir.ActivationFunctionType.Exp

    def mmdr(ps, lhsT, rhs, start, stop):
        nc.tensor.matmul(ps, lhsT, rhs, start=start, stop=stop, perf_mode=DR)

    with TileContext(nc) as tc:
        with (
            tc.tile_pool(name="wp", bufs=1) as wp,
            tc.tile_pool(name="act", bufs=2) as act,
            tc.tile_pool(name="att", bufs=4) as att,
            tc.tile_pool(name="outp", bufs=4) as outp,
            tc.tile_pool(name="psA", bufs=4, space="PSUM") as psA,
            tc.tile_pool(name="psB", bufs=2, space="PSUM") as psB,
            tc.tile_pool(name="psC", bufs=2, space="PSUM") as psC,
        ):
            W1 = wp.tile([128, 4, 2, 512], F8)
            QW = wp.tile([128, 2, 2, 512], F8)
            KW = wp.tile([128, 2, 2, 512], F8)
            VW = wp.tile([128, 2, 2, 512], F8)
            W3 = wp.tile([128, 2, 2, 2048], F8)
            WSH = wp.tile([128, 4, 2, 2048], F8)
            WSL = wp.tile([128, 4, 2, 2048], F8)
            PR = wp.tile([128, 4, 2, NT], F8)    # plane0: q-hat, plane1: pos
            ON8 = wp.tile([128, 2, 16], F8)
            ONMF = wp.tile([1, 128], F32)
            ONM = wp.tile([1, 128], F32R)
            ESH = wp.tile([128, 1], F32)
            ESC = wp.tile([128, 1], F32)
            B1S = wp.tile([128, 4], F32)
            B1B = wp.tile([128, 4], F32)
            QS = wp.tile([128, 4], F32)
            QB8 = wp.tile([128, 4], F32)
            KS = wp.tile([128, 4], F32)
            O2S = wp.tile([128, 4], F32)
            O2B = wp.tile([128, 4], F32)
            FS = wp.tile([128, 16], F32)
            FB = wp.tile([128, 16], F32)
            UVT = wp.tile([128, 1], F32)

            nc.vector.memset(ON8, 1.0)
            nc.vector.memset(ONMF, 1.0)
            nc.vector.tensor_copy(out=ONM, in_=ONMF)
            nc.vector.memset(ESH, EXPSHIFT)
            nc.vector.memset(ESC, EXPSCALE)

            def load_initial_weights():
                for k, t in [("b1s", B1S), ("b1b", B1B), ("qs", QS),
                             ("qb8", QB8), ("ks", KS), ("o2s", O2S),
                             ("o2b", O2B), ("fs", FS), ("fb", FB),
                             ("uvt", UVT)]:
                    nc.sync.dma_start(out=t, in_=d[k][:])
                nc.sync.dma_start(out=W1, in_=d["w1"][:])
                nc.sync.dma_start(out=QW, in_=d["qw"][:])
                nc.sync.dma_start(out=KW, in_=d["kw"][:])
                nc.sync.dma_start(out=VW, in_=d["vw"][:])
                for h in range(4):
                    nc.sync.dma_start(out=PR[:, h, 1, :], in_=d["pos"][:, h, :])

            def load_big_weights():
                for g in range(4):
                    sl = slice(g * 512, (g + 1) * 512)
                    nc.sync.dma_start(out=W3[:, :, :, sl],
                                      in_=d["w3"][:, :, :, sl])
                    nc.sync.dma_start(out=WSH[:, :, :, sl],
                                      in_=d["wsh"][:, :, :, sl])
                    nc.sync.dma_start(out=WSL[:, :, :, sl],
                                      in_=d["wsl"][:, :, :, sl])

            def trunk(e):
                """x DMA + conv1 + q/k/v projections + logits/exp for elem e.
                Returns tiles needed by attn/final phases."""
                XH = act.tile([128, 4, 2, NT], F8, tag="xh", name=f"xh{e}")
                XL = act.tile([128, 4, 2, NT], F8, tag="xl", name=f"xl{e}")
                nc.sync.dma_start(out=XH, in_=d["xh"][e])
                nc.sync.dma_start(out=XL, in_=d["xl"][e])
                if e == 0:
                    load_big_weights()

                # conv1 + bn1 + relu -> O1 fp8 (channels as [jj, i] pairs)
                O1 = act.tile([128, 2, 2, NT], F8, tag="o1", name=f"o1_{e}")
                for m in range(4):
                    ps = psA.tile([128, NT], F32, tag="mm")
                    for jj in range(4):
                        mmdr(ps, W1[:, jj, :, m * 128:(m + 1) * 128],
                             XH[:, jj], jj == 0, jj == 3)
                    nc.scalar.activation(O1[:, m // 2, m % 2, :], ps, RELU,
                                         bias=B1B[:, m:m + 1],
                                         scale=B1S[:, m:m + 1])

                # q projection: raw into KQ plane1, biased q-hat into PR pl0
                KQ = act.tile([128, 4, 2, NT], F8, tag="kq", name=f"kq{e}")
                for h in range(4):
                    ps = psA.tile([128, NT], F32, tag="mm")
                    for jj in range(2):
                        mmdr(ps, QW[:, jj, :, h * 128:(h + 1) * 128],
                             O1[:, jj], jj == 0, jj == 1)
                    nc.vector.tensor_scalar_mul(out=KQ[:, h, 1, :], in0=ps,
                                                scalar1=QS[:, h:h + 1])
                    nc.vector.tensor_scalar(out=PR[:, h, 0, :], in0=ps,
                                            scalar1=QS[:, h:h + 1],
                                            scalar2=QB8[:, h:h + 1],
                                            op0=MULT, op1=ADD)

                # k projection (no bias needed) into KQ plane0
                for h in range(4):
                    ps = psA.tile([128, NT], F32, tag="mm")
                    for jj in range(2):
                        mmdr(ps, KW[:, jj, :, h * 128:(h + 1) * 128],
                             O1[:, jj], jj == 0, jj == 1)
                    nc.vector.tensor_scalar_mul(out=KQ[:, h, 0, :], in0=ps,
                                                scalar1=KS[:, h:h + 1])

                # v, transposed: VT[tok, mt, c] (per-channel svw rides along)
                VT = act.tile([128, 2, 512], F8, tag="vt", name=f"vt{e}")
                for mt in range(2):
                    for cc in range(2):
                        ps = psA.tile([128, NT], F32, tag="mm")
                        for jj in range(2):
                            mmdr(ps, O1[:, jj, :, mt * 128:(mt + 1) * 128],
                                 VW[:, jj, :, cc * 256:(cc + 1) * 256],
                                 jj == 0, jj == 1)
                        nc.vector.tensor_scalar_mul(
                            out=VT[:, mt, cc * 256:(cc + 1) * 256], in0=ps,
                            scalar1=UVT)

                # logits^T + exp: one DR matmul per (h, mt)
                EXT = att.tile([128, 4, 2, NT], F8, tag="ext", bufs=2,
                               name=f"ext{e}")
                for h in range(4):
                    for mt in range(2):
                        psl = psA.tile([128, NT], F32, tag="mm")
                        mmdr(psl, KQ[:, h, :, mt * 128:(mt + 1) * 128],
                             PR[:, h], True, True)
                        nc.scalar.activation(EXT[:, h, mt, :], psl, EXPF,
                                             bias=ESH, scale=ESC)
                return XH, XL, VT, EXT

            def final_chunk(st, ms):
                """conv3 + compensated shortcut for m-tiles ms of element
                st = (e, XH, XL, O2)."""
                if st is None:
                    return
                e, XH, XL, O2 = st
                for m in ms:
                    sl = slice(m * 128, (m + 1) * 128)
                    ps = psA.tile([128, NT], F32, tag="mm")
                    for jj in range(4):
                        mmdr(ps, WSH[:, jj, :, sl], XH[:, jj], jj == 0, False)
                    for jj in range(4):
                        mmdr(ps, WSL[:, jj, :, sl], XH[:, jj], False, False)
                    for jj in range(4):
                        mmdr(ps, WSH[:, jj, :, sl], XL[:, jj], False, False)
                    for jj in range(2):
                        mmdr(ps, W3[:, jj, :, sl], O2[:, jj], False, jj == 1)
                    ot = outp.tile([128, NT], F32, tag="ot")
                    nc.scalar.activation(ot, ps, RELU, bias=FB[:, m:m + 1],
                                         scale=FS[:, m:m + 1])
                    nc.sync.dma_start(out=out_d[e, m], in_=ot)

            def attn(e, VT, EXT, prev):
                """softmax + attn@V for elem e, interleaved with the final
                conv of the previous element (fills PE dependency gaps)."""
                O2 = act.tile([128, 2, 2, NT], F8, tag="o2", name=f"o2_{e}")
                for h in range(4):
                    pss = psC.tile([1, NT], F32, tag="sum")
                    mmdr(pss, ON8[:, :, 0:1], EXT[:, h], True, True)
                    rec = att.tile([1, NT], F32R, tag="rec", bufs=4,
                                   name=f"rec{e}_{h}")
                    with nc.allow_low_precision(reason="softmax 1/sum"):
                        nc.vector.reciprocal(out=rec, in_=pss)
                    final_chunk(prev, range(4 * h, 4 * h + 2))
                    psr = psB.tile([128, NT], F32, tag="rcb")
                    nc.tensor.matmul(psr, ONM, rec, start=True, stop=True)
                    rcb = att.tile([128, NT], F32, tag="rcbs", bufs=4,
                                   name=f"rcb{e}_{h}")
                    nc.vector.tensor_copy(out=rcb, in_=psr)
                    final_chunk(prev, range(4 * h + 2, 4 * h + 4))
                    pso = psA.tile([128, NT], F32, tag="mm")
                    mmdr(pso, VT[:, :, h * 128:(h + 1) * 128], EXT[:, h],
                         True, True)
                    tmp = att.tile([128, NT], F32, tag="tmp", bufs=2,
                                   name=f"tmp{e}_{h}")
                    nc.vector.tensor_tensor(out=tmp, in0=pso, in1=rcb,
                                            op=MULT)
                    nc.scalar.activation(O2[:, h // 2, h % 2, :], tmp, RELU,
                                         bias=O2B[:, h:h + 1],
                                         scale=O2S[:, h:h + 1])
                return O2

            load_initial_weights()
            prev = None
            for e in range(BLOC):
                XH, XL, VT, EXT = trunk(e)
                O2 = attn(e, VT, EXT, prev)
                prev = (e, XH, XL, O2)
            final_chunk(prev, range(16))

    nc.compile()
    return nc


def _q8(x):
    """quantize to fp8e4 values, returned as float32."""
    v = np.asarray(x, np.float32).astype(E4)
    return v.astype(np.float32)


def _r8(wq):
    """quantized [M, K] weight (f32 container) -> lhsT/moving layout
    [128, K//256, 2, M] fp8."""
    m, k = wq.shape
    t = np.ascontiguousarray(
        wq.T.reshape(k // 256, 2, 128, m).transpose(2, 0, 1, 3))
    return t.astype(E4)


def _b(v):
    """[C] vector -> [128, C//128] per-m-tile layout."""
    return np.ascontiguousarray(
        np.asarray(v, np.float64).reshape(-1, 128).T).astype(np.float32)


def _prep_shared(i):
    s1 = (i["bn1_g"] / np.sqrt(i["bn1_v"] + EPS)).astype(np.float64)
    w1f = i["conv1_w"].astype(np.float64) * s1[:, None]
    b1 = i["bn1_b"].astype(np.float64) - i["bn1_m"].astype(np.float64) * s1

    s2 = (i["bn2_g"] / np.sqrt(i["bn2_v"] + EPS)).astype(np.float64)
    b2 = (i["bn2_b"].astype(np.float64)
          - i["bn2_m"].astype(np.float64) * s2
          + s2 * i["v_b"].astype(np.float64))   # v bias folded (probs sum 1)

    s3 = (i["bn3_g"] / np.sqrt(i["bn3_v"] + EPS)).astype(np.float64)
    w3f = i["conv3_w"].astype(np.float64) * s3[:, None]
    b3 = i["bn3_b"].astype(np.float64) - i["bn3_m"].astype(np.float64) * s3

    ss = (i["scbn_g"] / np.sqrt(i["scbn_v"] + EPS)).astype(np.float64)
    wscf = i["sc_w"].astype(np.float64) * ss[:, None]
    bsc = (ss * (i["sc_b"].astype(np.float64)
                 - i["scbn_m"].astype(np.float64))
           + i["scbn_b"].astype(np.float64))

    sw1 = SAFE * MAXV / np.abs(w1f).max(axis=1)
    sqw = SAFE * MAXV / np.abs(i["q_w"]).max(axis=1)
    skw = SAFE * MAXV / np.abs(i["k_w"]).max(axis=1)
    svw = SAFE * MAXV / np.abs(i["v_w"]).max(axis=1)
    c_ch = SAFE * MAXV / np.maximum(np.abs(w3f).max(axis=1) / SO2,
                                    np.abs(wscf).max(axis=1) / SX)
    uv = 32.0 / (SO1 * np.median(svw))

    wscs = wscf * (c_ch / SX)[:, None]
    wsch_f = _q8(wscs)

    pos = (np.asarray(i["rel_h"], np.float64)
           + np.asarray(i["rel_w"], np.float64)).reshape(4, 128, NT)
    pos8 = (pos * SKP).astype(np.float32).astype(E4)

    return {
        "w1": _r8(_q8(w1f * sw1[:, None])),
        "qw": _r8(_q8(i["q_w"] * sqw[:, None])),
        "kw": _r8(_q8(i["k_w"] * skw[:, None])),
        "vw": _r8(_q8(i["v_w"] * svw[:, None])),
        "w3": _r8(_q8(w3f * (c_ch / SO2)[:, None])),
        "wsh": _r8(wsch_f),
        "wsl": _r8(_q8(wscs - wsch_f)),
        "pos": np.ascontiguousarray(pos8.transpose(1, 0, 2)),
        "b1s": _b(1.0 / (sw1 * SX) * SO1), "b1b": _b(b1 * SO1),
        "qs": _b(SQ / (sqw * SO1)), "qb8": _b(SQ * i["q_b"]),
        "ks": _b(SKP / (skw * SO1)),
        "o2s": _b(s2 * SO2 / (svw * SO1 * uv)), "o2b": _b(b2 * SO2),
        "fs": _b(1.0 / c_ch), "fb": _b(b3 + bsc),
        "uvt": np.full((128, 1), uv, np.float32),
    }


def _prep_x(x):
    """full x [64, 1024, 16, 16] -> per-core hi/lo fp8 [8, BLOC,128,4,2,NT]."""
    xs = np.asarray(x, np.float32).reshape(64, 1024, NT) * SX
    xh = xs.astype(E4)
    xl = (xs - xh.astype(np.float32)).astype(E4)

    def lay(a):
        t = a.reshape(64, 4, 2, 128, NT).transpose(0, 3, 1, 2, 4)
        return np.ascontiguousarray(t).reshape(8, BLOC, 128, 4, 2, NT)

    return lay(xh), lay(xl)


def kernel(**inputs):
    if "nc" not in _STATE:
        _STATE["nc"] = _build_nc()
    nc = _STATE["nc"]

    i = {k: np.asarray(v) for k, v in inputs.items()}
    shared = _prep_shared(i)
    xh, xl = _prep_x(i["x"])

    in_maps = []
    for c in range(NCORES):
        m = dict(shared)
        m["xh"] = xh[c]
        m["xl"] = xl[c]
        in_maps.append(m)

    res = run_bass_kernel_spmd(nc, in_maps, list(range(NCORES)))
    out = np.concatenate(
        [res.results[c]["out"].reshape(BLOC, 2048, 16, 16)
         for c in range(NCORES)], axis=0)
    return out.astype(np.float32)
